# revision 25
# baseline (speedup 1.0000x reference)
"""3-layer GCN + linear head on 8 TRN2 NeuronCores (Bass/Tile, SPMD).

Self-contained: hardcodes N=50000, E=600000, D=128, DOUT=32, 8 cores.

Math (matches the reference):
    src,dst + self-loops; deg = in-degree incl self-loop; dinv = rsqrt(deg)
    norm_e = dinv[src]*dinv[dst]
    layer(h): agg[d] = sum_e norm_e (hW)[src_e]; relu(agg+b)
    out = h3 @ Wl + bl

v2 design:
- Nodes sharded into 8 slabs (graph parallel); per layer Z~ = dinv (H W) is
  written as an fp8e4m3 DUPLICATED-row slab ([row;row] = 256B, the SWDGE
  gather minimum) and AllGathered into a shared table per half.
- Self-loops are NOT in the edge stream: Z~_local is added by DVE directly.
- Edges (w/o loops) are packed host-side into 128-edge chunks per
  (dst-block, src-half) with a SHARED even chunk-count profile; per-core
  cap-constrained bin-packing of nodes into blocks keeps padding ~1-3%.
- Gather: 1024-edge SWDGE groups (ring limit) of 256B fp8 rows, 4 queues.
- Aggregation: fp8 DoubleRow matmuls — 256 edges (2 chunks) per PE
  instruction: lhsT = one-hot sel [128,2,128] fp8 (DVE is_equal), rhs = msg
  [128,2,128] fp8 -> PSUM f32. Bias enters as a f16 rank-1 matmul
  (recipd x b); dinv scaling + relu on Act.
- Two passes per layer over the 49 blocks: pass1 accumulates stream-0
  chunks and spills acc_b = PSUM + Z~_local(self-loop) to SBUF f16 (DVE);
  pass2 accumulates stream-1 chunks, DVE-adds acc_b, closes with the bias
  matmul, then relu -> PE transpose -> next layer's Z~ per block. AllGathers
  fire when half-slabs complete, hidden behind the opposite pass.
- idx/dloc tiles are preloaded once (shared by all 3 layers).
"""
import sys
sys.path.insert(0, '/opt/trn_rl_repo')
import numpy as np
import ml_dtypes

import concourse.bass as bass
import concourse.tile as tile
import concourse.mybir as mybir
from concourse import bacc
from concourse.library_config import mlp as mlp_lib

P = 128
GROUP = 768           # edges per dma_gather (1024-desc ring: 1.33 per queue)
GCH = GROUP // P      # chunk-cols per group
GPR = GROUP // 256    # pair-chunks per group
SB = 6                # gather groups per sel batch
NQ = 4                # SWDGE queues
LOOKAHEAD = 8         # gather groups in flight per stream phase
ZB = 8                # z~ blocks batched per slab DMA

N_NODES = 50000
N_CORES = 8
DIN = 128
DOUT = 32

np8 = ml_dtypes.float8_e4m3


def _profile_counts(maxq, nblk, slack):
    """Even per-block 128-chunk counts covering maxq+slack edges."""
    need = maxq + slack
    base = max(2, 2 * (need // (256 * nblk)))
    while nblk * base * P > need:
        base -= 2
    base = max(2, base)
    rem = need - nblk * base * P
    ups = min(nblk, (rem + 255) // 256) if rem > 0 else 0
    cnt = np.full(nblk, base, np.int64)
    cnt[:ups] += 2
    return cnt


def _pack_core(indeg, caps, nblk, nodes):
    """Assign `nodes` (list of local ids) to nblk bins of 128 slots with
    per-stream edge caps caps[b][st]. Returns slot_of dict or None."""
    d0 = indeg[nodes, 0].astype(np.int64)
    d1 = indeg[nodes, 1].astype(np.int64)
    order = np.argsort(-(d0 + d1), kind="stable")
    slots_left = np.full(nblk, P, np.int64)
    caps = caps.astype(np.float64)
    used = np.zeros_like(caps)                      # [nblk, 2]
    assign = np.empty(len(nodes), np.int64)
    for i in order:
        dd = np.array([d0[i], d1[i]], np.float64)
        ok = (slots_left > 0) & np.all(caps - used >= dd, axis=1)
        if not ok.any():
            return None
        # most-balanced fill: minimize the worse stream's fill ratio
        ratio = np.max((used + dd) / caps, axis=1)
        ratio[~ok] = 1e18
        b = int(np.argmin(ratio))
        assign[i] = b
        slots_left[b] -= 1
        used[b] += dd
    slot_of = np.empty(len(nodes), np.int64)
    fill = np.zeros(nblk, np.int64)
    for i in range(len(nodes)):
        b = assign[i]
        slot_of[i] = b * P + fill[b]
        fill[b] += 1
    return slot_of


def _preprocess(edge_index, N, C):
    LOCAL = N // C
    NB = (LOCAL + P - 1) // P
    PADL = NB * P
    H0B = NB // 2
    H1B = NB - H0B
    H0R, H1R = H0B * P, H1B * P
    TOT0, TOT1 = C * H0R, C * H1R
    assert TOT0 <= 32768 and TOT1 <= 32768

    src = np.asarray(edge_index[0], dtype=np.int64)
    dst = np.asarray(edge_index[1], dtype=np.int64)

    # degrees INCLUDE self-loops (reference semantics)
    deg = (np.bincount(dst, minlength=N) + 1).astype(np.float64)
    dinv = (1.0 / np.sqrt(deg)).astype(np.float32)
    sdeg = np.sqrt(deg).astype(np.float32)

    score = src // LOCAL
    slocal = src % LOCAL
    s_st = (slocal >= H0R).astype(np.int64)   # source half = stream
    core = dst // LOCAL
    ldst = dst % LOCAL

    # per-core per-node in-degree by stream (excl self-loops)
    indeg = np.zeros((C, LOCAL, 2), np.int64)
    for c in range(C):
        m = core == c
        for st in (0, 1):
            cnts = np.bincount(ldst[m & (s_st == st)], minlength=LOCAL)
            indeg[c, :, st] = cnts

    # shared chunk-count profile per (half, stream) quadrant
    for slack in (256, 512, 1024, 2048):
        cnt = np.zeros((NB, 2), np.int64)
        for h, (lo, b0, nb) in enumerate([(0, 0, H0B), (H0R, H0B, H1B)]):
            hi = lo + (H0R if h == 0 else LOCAL - H0R)
            for st in (0, 1):
                maxq = max(int(indeg[c, lo:hi, st].sum()) for c in range(C))
                cnt[b0:b0 + nb, st] = _profile_counts(maxq, nb, slack)
        # pad each stream's total chunk count to a multiple of GCH
        for st in (0, 1):
            t = int(cnt[:, st].sum())
            i = 0
            while t % GCH:
                cnt[i % NB, st] += 2
                t += 2
                i += 1
        # per-core packing
        slot_of = np.zeros((C, LOCAL), np.int64)
        ok = True
        for c in range(C):
            for h, (lo, b0, nb) in enumerate([(0, 0, H0B), (H0R, H0B, H1B)]):
                hi = lo + (H0R if h == 0 else LOCAL - H0R)
                nodes = np.arange(lo, hi)
                res = _pack_core(indeg[c], cnt[b0:b0 + nb] * P, nb, nodes)
                if res is None:
                    ok = False
                    break
                slot_of[c, nodes] = b0 * P + res
            if not ok:
                break
        if ok:
            break
    assert ok, "bin packing failed at max slack"

    # perm: slot -> orig local id (-1 pad).  half1 has H1R - (LOCAL-H0R)
    # pad slots; they end up wherever packing left slots unused -> compute
    perm = np.full((C, PADL), -1, np.int64)
    for c in range(C):
        perm[c][slot_of[c]] = np.arange(LOCAL)

    # table ids of sources (per the SOURCE core's permutation)
    sslot = slot_of[score, slocal]
    sid = np.where(s_st == 0, score * H0R + sslot,
                   score * H1R + (sslot - H0R))

    n_chunks = [int(cnt[:, st].sum()) for st in (0, 1)]
    n_groups = [n // GCH for n in n_chunks]

    idx_all, dloc_all = [], []
    for c in range(C):
        m = core == c
        cl = slot_of[c][ldst[m]]
        cs, cst = sid[m], s_st[m]
        blk = cl // P
        dl = cl % P
        per_idx, per_dl = [], []
        for st in (0, 1):
            idx_st = np.zeros(n_chunks[st] * P, np.int64)
            dl_st = -np.ones(n_chunks[st] * P, np.int64)
            pos = 0
            for b in range(NB):
                sm = (blk == b) & (cst == st)
                ii, dd = cs[sm], dl[sm]
                want = cnt[b, st] * P
                assert len(ii) <= want, (c, b, st, len(ii), want)
                idx_st[pos:pos + len(ii)] = ii
                dl_st[pos:pos + len(ii)] = dd
                pos += want
            # wrap idx per 1024-group: [G, 128, 64] -> [128, G*64]
            g = n_groups[st]
            w = idx_st.reshape(g, GROUP // 16, 16)
            w = np.transpose(w, (0, 2, 1))             # [g, 16, 64]
            w = np.tile(w, (1, 8, 1))                  # [g, 128, 64]
            per_idx.append(np.ascontiguousarray(
                np.transpose(w, (1, 0, 2)).reshape(P, -1)).astype(np.int16))
            # dloc pairs: [U, 2, 128] -> [128, U*2] f16
            u = n_chunks[st] // 2
            d = dl_st.reshape(u, 2, P)
            per_dl.append(np.ascontiguousarray(
                np.transpose(d, (2, 0, 1)).reshape(P, -1)).astype(np.float16))
        idx_all.append(per_idx)
        dloc_all.append(per_dl)

    return dict(
        LOCAL=LOCAL, NB=NB, PADL=PADL, C=C,
        H0B=H0B, H1B=H1B, TOT0=TOT0, TOT1=TOT1,
        cnt=cnt, n_chunks=n_chunks, n_groups=n_groups,
        idx_all=idx_all, dloc_all=dloc_all, dinv=dinv, sdeg=sdeg, perm=perm,
    )


def _host_tensors(pp, x, weights):
    C, LOCAL, PADL, NB = pp["C"], pp["LOCAL"], pp["PADL"], pp["NB"]
    W1, b1, W2, b2, W3, b3, Wl, bl = weights
    iota = np.tile(np.arange(P, dtype=np.float16), (P, 1))
    ident = np.eye(P, dtype=np.float16)
    ones = np.ones((1, P), np.float16)
    maps = []
    for c in range(C):
        pm = pp["perm"][c]
        valid = pm >= 0
        pmv = np.where(valid, pm, 0)
        xs = np.where(valid[:, None], x[c * LOCAL + pmv], 0).astype(np.float32)
        dvl = np.where(valid, pp["dinv"][c * LOCAL + pmv], 0).astype(np.float32)
        dv = np.ascontiguousarray(dvl.reshape(NB, P).T)
        rd = np.where(valid, pp["sdeg"][c * LOCAL + pmv], 0
                      ).astype(np.float16).reshape(1, -1)
        m = {
            "xt": np.ascontiguousarray(xs.T).astype(np.float16),
            "w1": np.ascontiguousarray(W1, np.float16),
            "w2": np.ascontiguousarray(W2, np.float16),
            "w3": np.ascontiguousarray(W3, np.float16),
            "wl": np.ascontiguousarray(Wl, np.float16),
            "b1": np.asarray(b1, np.float16).reshape(1, -1),
            "b2": np.asarray(b2, np.float16).reshape(1, -1),
            "b3": np.asarray(b3, np.float16).reshape(1, -1),
            "bl": np.asarray(bl, np.float16).reshape(1, -1),
            "dinv_sb": dv, "recipd": rd,
            "iota": iota, "ident": ident, "ones": ones,
        }
        for st in (0, 1):
            m[f"idx{st}"] = pp["idx_all"][c][st]
            m[f"dloc{st}"] = pp["dloc_all"][c][st]
        maps.append(m)
    return maps


def _build(pp, DOUT_, n_cores):
    NB, PADL = pp["NB"], pp["PADL"]
    H0B, H1B, TOT0, TOT1 = pp["H0B"], pp["H1B"], pp["TOT0"], pp["TOT1"]
    cnt, n_groups = pp["cnt"], pp["n_groups"]
    f32 = mybir.dt.float32
    f16 = mybir.dt.float16
    f8 = mybir.dt.float8e4

    nc = bacc.Bacc("TRN2", target_bir_lowering=False, debug=False,
                   num_devices=n_cores, num_swdge_queues=NQ)

    xt = nc.dram_tensor("xt", [P, PADL], f16, kind="ExternalInput")
    w = [nc.dram_tensor(f"w{i+1}", [P, P], f16, kind="ExternalInput") for i in range(3)]
    wl = nc.dram_tensor("wl", [P, DOUT_], f16, kind="ExternalInput")
    bias = [nc.dram_tensor(f"b{i+1}", [1, P], f16, kind="ExternalInput") for i in range(3)]
    bl = nc.dram_tensor("bl", [1, DOUT_], f16, kind="ExternalInput")
    dinv_sb_d = nc.dram_tensor("dinv_sb", [P, NB], f32, kind="ExternalInput")
    recipd_d = nc.dram_tensor("recipd", [1, PADL], f16, kind="ExternalInput")
    iota_d = nc.dram_tensor("iota", [P, P], f16, kind="ExternalInput")
    ident_d = nc.dram_tensor("ident", [P, P], f16, kind="ExternalInput")
    ones_d = nc.dram_tensor("ones", [1, P], f16, kind="ExternalInput")
    idx_d, dloc_d = [None, None], [None, None]
    for st in (0, 1):
        idx_d[st] = nc.dram_tensor(f"idx{st}", [P, n_groups[st] * (GROUP // 16)],
                                   mybir.dt.int16, kind="ExternalInput")
        dloc_d[st] = nc.dram_tensor(f"dloc{st}", [P, n_groups[st] * GPR * 2],
                                    f16, kind="ExternalInput")
    out_d = nc.dram_tensor("out", [PADL, DOUT_], f32, kind="ExternalOutput")

    rg = [list(range(n_cores))]

    from contextlib import ExitStack
    with tile.TileContext(nc) as tc, ExitStack() as ctx:
        dram = ctx.enter_context(tc.tile_pool(name="dram", bufs=1, space="DRAM"))
        cpool = ctx.enter_context(tc.tile_pool(name="consts", bufs=1))
        hpool = ctx.enter_context(tc.tile_pool(name="ht", bufs=1))
        mpool = ctx.enter_context(tc.tile_pool(name="msg", bufs=28))
        spool = ctx.enter_context(tc.tile_pool(name="sel", bufs=3))
        zpool = ctx.enter_context(tc.tile_pool(name="hn", bufs=3))
        opool = ctx.enter_context(tc.tile_pool(name="outs", bufs=3))
        pz = ctx.enter_context(tc.tile_pool(name="pz", bufs=2, space="PSUM"))
        pa = ctx.enter_context(tc.tile_pool(name="pa", bufs=4, space="PSUM"))
        pt = ctx.enter_context(tc.tile_pool(name="pt", bufs=1, space="PSUM"))

        nc.gpsimd.load_library(mlp_lib)

        def const(dram_t, shape, dt=f16):
            t = cpool.tile(shape, dt, name=dram_t.name + "_sb")
            nc.sync.dma_start(t[:], dram_t[:])
            return t
        w_sb = [const(w[i], [P, P]) for i in range(3)]
        wl_sb = const(wl, [P, DOUT_])
        b_sb = [const(bias[i], [1, P]) for i in range(3)]
        bl_sb = const(bl, [1, DOUT_])
        dinv_sb = const(dinv_sb_d, [P, NB], f32)
        recipd = const(recipd_d, [1, PADL])
        iota = const(iota_d, [P, P])
        ident = const(ident_d, [P, P])
        ones = const(ones_d, [1, P])
        idx_sb = [const(idx_d[st], [P, n_groups[st] * (GROUP // 16)],
                        mybir.dt.int16) for st in (0, 1)]
        dloc_sb = [const(dloc_d[st], [P, n_groups[st] * GPR * 2])
                   for st in (0, 1)]

        # persistent state
        hT = [hpool.tile([P, PADL], f16, name=f"hT{i}") for i in range(2)]
        nc.sync.dma_start(hT[0][:], xt[:])
        z8 = [hpool.tile([P, NB * 256], f8, name=f"z8_{i}") for i in range(2)]
        acc = hpool.tile([P, PADL], f16, name="acc")

        slab = [[dram.tile([H0B * P, 256], f8, name=f"slab0_{i}"),
                 dram.tile([H1B * P, 256], f8, name=f"slab1_{i}")]
                for i in range(3)]
        table = [[dram.tile([TOT0, 256], f8, addr_space="Shared",
                            name=f"table0_{i}"),
                  dram.tile([TOT1, 256], f8, addr_space="Shared",
                            name=f"table1_{i}")]
                 for i in range(3)]

        gq = [0]

        zstate = {}

        def z_block(layer, hTsrc, b):
            """Z~ matmul for block b of `layer` -> fp8 dup into z8[layer%2];
            slab DMA every ZB blocks (not straddling the half boundary)."""
            zp = pz.tile([P, P], f32, tag="z")
            nc.tensor.matmul(zp[:], lhsT=hTsrc[:, b * P:(b + 1) * P],
                             rhs=w_sb[layer][:], start=True, stop=True)
            zcur = z8[layer % 2]
            nc.scalar.activation(
                zcur[:, b * 256:(b + 1) * 256].rearrange("p (j e) -> p j e", e=P),
                zp[:].unsqueeze(1).to_broadcast([P, 2, P]),
                mybir.ActivationFunctionType.Copy,
                scale=dinv_sb[:, b:b + 1])
            half = 0 if b < H0B else 1
            bb = b if half == 0 else b - H0B
            nblk = H0B if half == 0 else H1B
            if bb % ZB == 0:
                zstate["b0"] = bb
            if bb % ZB == ZB - 1 or bb == nblk - 1:
                b0 = zstate["b0"]
                nb_ = bb - b0 + 1
                base = (b0 if half == 0 else H0B + b0)
                nc.sync.dma_start(
                    slab[layer][half][b0 * P:(b0 + nb_) * P, :].rearrange(
                        "(g p) f -> p g f", p=P),
                    zcur[:, base * 256:(base + nb_) * 256].rearrange(
                        "p (g f) -> p g f", f=256),
                )

        def ag(layer, half):
            nc.gpsimd.collective_compute(
                "AllGather", mybir.AluOpType.bypass, replica_groups=rg,
                ins=[slab[layer][half].opt()], outs=[table[layer][half].opt()],
            )

        # gather state shared across layers so the NEXT layer's stream-0
        # groups can be pumped during the current layer's tail (hides the
        # gather pipeline refill at layer boundaries)
        g_msg = {}
        g_next = {(l, st): 0 for l in range(3) for st in (0, 1)}

        def pump(layer, st):
            g = g_next[(layer, st)]
            if g >= n_groups[st]:
                return
            g_next[(layer, st)] = g + 1
            mt = mpool.tile([P, GCH * 256], f8, tag="msg")
            nc.gpsimd.dma_gather(
                out_ap=mt[:].rearrange("p (c e) -> p c e", e=256),
                in_ap=table[layer][st][:],
                idxs_ap=idx_sb[st][:, g * (GROUP // 16):(g + 1) * (GROUP // 16)],
                num_idxs=GROUP,
                num_idxs_reg=GROUP,
                elem_size=256,
                queue_num=gq[0] % NQ,
            )
            gq[0] += 1
            g_msg[(layer, st, g)] = mt

        # --- per-layer aggregation ---------------------------------------
        def agg_layer(layer, hTcur, hTnext):
            zcur = z8[layer % 2]

            sel_tiles = [{}, {}]

            def ensure_batch(st, t):
                if t in sel_tiles[st]:
                    return sel_tiles[st][t]
                u0 = t * SB * GPR
                nu = min(SB * GPR, n_groups[st] * GPR - u0)
                st_t = spool.tile([P, SB * GPR * 256], f8, tag="sel")
                nc.vector.tensor_tensor(
                    out=st_t[:, :nu * 256].rearrange(
                        "p (u j s) -> p u j s", j=2, s=P),
                    in0=dloc_sb[st][:, u0 * 2:(u0 + nu) * 2].rearrange(
                        "p (u j) -> p u j", j=2).unsqueeze(3).to_broadcast(
                        [P, nu, 2, P]),
                    in1=iota[:].unsqueeze(1).unsqueeze(1).to_broadcast(
                        [P, nu, 2, P]),
                    op=mybir.AluOpType.is_equal,
                )
                sel_tiles[st][t] = st_t
                return st_t

            cursor = [0, 0]     # pair-chunk cursor per stream

            def emit_pairs(st, b, ap, first):
                """DoubleRow matmuls for block b's stream-st pairs."""
                for _ in range(int(cnt[b, st]) // 2):
                    u = cursor[st]
                    cursor[st] += 1
                    g, j2 = u // GPR, u % GPR
                    mt = g_msg[(layer, st, g)]
                    selt = ensure_batch(st, u // (SB * GPR))
                    k = u - (u // (SB * GPR)) * SB * GPR
                    nc.tensor.matmul(
                        ap[:],
                        lhsT=selt[:, k * 256:(k + 1) * 256].rearrange(
                            "p (j s) -> p j s", s=P),
                        rhs=mt[:].rearrange("p (c e) -> p c e", e=256)[
                            :, 2 * j2:2 * j2 + 2, 0:P],
                        start=first, stop=False,
                        perf_mode=mybir.MatmulPerfMode.DoubleRow,
                    )
                    first = False
                    if j2 == GPR - 1:
                        pump(layer, st)  # group fully claimed; prefetch next
                    if k == SB * GPR - 1:   # last pair of sel batch
                        t = u // (SB * GPR)
                        if (t + 1) * SB * GPR < n_groups[st] * GPR:
                            ensure_batch(st, t + 1)
                return first

            # pass 1: stream-0 partials, spilled (+ self-loop term) to acc
            while g_next[(layer, 0)] < min(LOOKAHEAD, n_groups[0]):
                pump(layer, 0)
            for b in range(NB):
                ap = pa.tile([P, P], f32, tag="agg")
                first = emit_pairs(0, b, ap, True)
                assert not first    # cnt >= 2 guarantees stream-0 pairs
                # acc_b = partial + Z~_local (self-loop), f16
                nc.vector.tensor_tensor(
                    out=acc[:, b * P:(b + 1) * P], in0=ap[:],
                    in1=zcur[:, b * 256:b * 256 + P],
                    op=mybir.AluOpType.add)

            # pass 2: stream-1 + close + activations + next-layer Z~
            while g_next[(layer, 1)] < min(LOOKAHEAD, n_groups[1]):
                pump(layer, 1)
            for b in range(NB):
                ap = pa.tile([P, P], f32, tag="agg")
                first = emit_pairs(1, b, ap, True)
                # close with rank-1 bias (recipd x b), f16
                nc.tensor.matmul(ap[:], lhsT=recipd[0:1, b * P:(b + 1) * P],
                                 rhs=b_sb[layer][:], start=first, stop=True)
                # += stream-0 partial (+self-loop)
                nc.vector.tensor_tensor(
                    out=ap[:], in0=ap[:], in1=acc[:, b * P:(b + 1) * P],
                    op=mybir.AluOpType.add)
                hn = zpool.tile([P, P], f16, tag="hn")
                nc.scalar.activation(hn[:], ap[:],
                                     mybir.ActivationFunctionType.Relu,
                                     scale=dinv_sb[:, b:b + 1])
                tp = pt.tile([P, P], f16, tag="tp")
                nc.tensor.transpose(out=tp[:], in_=hn[:], identity=ident[:])
                nc.vector.tensor_copy(hTnext[:, b * P:(b + 1) * P], tp[:])
                if layer < 2:
                    z_block(layer + 1, hTnext, b)
                    if b == H0B - 1:
                        ag(layer + 1, 0)
                    elif b == NB - 1:
                        ag(layer + 1, 1)
                else:
                    hp = pz.tile([P, DOUT_], f32, tag="z")
                    nc.tensor.matmul(hp[:], lhsT=hTnext[:, b * P:(b + 1) * P],
                                     rhs=wl_sb[:], start=True, stop=False)
                    nc.tensor.matmul(hp[:], lhsT=ones[:], rhs=bl_sb[:],
                                     start=False, stop=True)
                    ot = opool.tile([P, DOUT_], f32, tag="o")
                    nc.scalar.activation(ot[:], hp[:],
                                         mybir.ActivationFunctionType.Copy)
                    nc.sync.dma_start(out_d[b * P:(b + 1) * P, :], ot[:])

            # cross-layer warmup: issue the next layer's stream-0 gathers now
            # (they park on the AG0 sem while this layer's PE tail drains)
            if layer < 2:
                for _ in range(min(LOOKAHEAD, n_groups[0])):
                    pump(layer + 1, 0)

        # prologue: layer-0 Z~ from x, both AGs
        for b in range(NB):
            z_block(0, hT[0], b)
            if b == H0B - 1:
                ag(0, 0)
            elif b == NB - 1:
                ag(0, 1)

        for layer in range(3):
            agg_layer(layer, hT[layer % 2], hT[(layer + 1) % 2])

    nc.compile()
    return nc


_CACHE = {}


def _get_compiled(edge_index):
    key = hash(np.asarray(edge_index, np.int64).tobytes())
    if key not in _CACHE:
        pp = _preprocess(edge_index, N_NODES, N_CORES)
        nc = _build(pp, DOUT, N_CORES)
        _CACHE[key] = (pp, nc)
    return _CACHE[key]


_LAST_RUN = {}


def kernel(x, edge_index, W1, b1, W2, b2, W3, b3, Wl, bl):
    x = np.asarray(x, np.float32)
    pp, nc = _get_compiled(edge_index)
    maps = _host_tensors(pp, x, (W1, b1, W2, b2, W3, b3, Wl, bl))

    from concourse.bass_utils import run_bass_kernel_spmd
    res = run_bass_kernel_spmd(nc, maps, core_ids=list(range(N_CORES)))
    LOCAL = pp["LOCAL"]
    parts = []
    for c in range(N_CORES):
        r = np.asarray(res.results[c]["out"])
        pm = pp["perm"][c]
        valid = pm >= 0
        o = np.zeros((LOCAL, r.shape[1]), r.dtype)
        o[pm[valid]] = r[valid]
        parts.append(o)
    out = np.concatenate(parts)
    _LAST_RUN["nc"] = nc
    _LAST_RUN["maps"] = maps
    return out


def _install_ntff_hook():
    """The agent image's antenv lacks axon_hooks; recreate it from the boot
    helper so run_bass_kernel_spmd(trace=True) can capture NTFF profiles."""
    import types
    if "antenv.axon_hooks" in sys.modules:
        return
    mod = types.ModuleType("antenv.axon_hooks")
    _state = {}
    mod.set_axon_ntff_profile_hook = lambda h: _state.__setitem__("h", h)
    mod.get_axon_ntff_profile_hook = lambda: _state.get("h")
    sys.modules["antenv.axon_hooks"] = mod
    import antenv
    antenv.axon_hooks = mod
    from trn_agent_boot.trn_boot import _ntff_profile_via_ctypes
    mod.set_axon_ntff_profile_hook(
        _ntff_profile_via_ctypes("/opt/axon/libaxon_pjrt.so"))


def profile_exec_ns():
    """Re-run the last kernel invocation with NTFF tracing; return exec ns."""
    if "nc" not in _LAST_RUN:
        return None
    _install_ntff_hook()
    from concourse.bass_utils import run_bass_kernel_spmd
    res = run_bass_kernel_spmd(
        _LAST_RUN["nc"], _LAST_RUN["maps"],
        core_ids=list(range(N_CORES)), trace=True,
    )
    _LAST_RUN["trace_res"] = res
    return res.exec_time_ns


# revision 26
# speedup vs baseline: 1.0051x; 1.0051x over previous
"""3-layer GCN + linear head on 8 TRN2 NeuronCores (Bass/Tile, SPMD).

Self-contained: hardcodes N=50000, E=600000, D=128, DOUT=32, 8 cores.

Math (matches the reference):
    src,dst + self-loops; deg = in-degree incl self-loop; dinv = rsqrt(deg)
    norm_e = dinv[src]*dinv[dst]
    layer(h): agg[d] = sum_e norm_e (hW)[src_e]; relu(agg+b)
    out = h3 @ Wl + bl

v2 design:
- Nodes sharded into 8 slabs (graph parallel); per layer Z~ = dinv (H W) is
  written as an fp8e4m3 DUPLICATED-row slab ([row;row] = 256B, the SWDGE
  gather minimum) and AllGathered into a shared table per half.
- Self-loops are NOT in the edge stream: Z~_local is added by DVE directly.
- Edges (w/o loops) are packed host-side into 128-edge chunks per
  (dst-block, src-half) with a SHARED even chunk-count profile; per-core
  cap-constrained bin-packing of nodes into blocks keeps padding ~1-3%.
- Gather: 1024-edge SWDGE groups (ring limit) of 256B fp8 rows, 4 queues.
- Aggregation: fp8 DoubleRow matmuls — 256 edges (2 chunks) per PE
  instruction: lhsT = one-hot sel [128,2,128] fp8 (DVE is_equal), rhs = msg
  [128,2,128] fp8 -> PSUM f32. Bias enters as a f16 rank-1 matmul
  (recipd x b); dinv scaling + relu on Act.
- Two passes per layer over the 49 blocks: pass1 accumulates stream-0
  chunks and spills acc_b = PSUM + Z~_local(self-loop) to SBUF f16 (DVE);
  pass2 accumulates stream-1 chunks, DVE-adds acc_b, closes with the bias
  matmul, then relu -> PE transpose -> next layer's Z~ per block. AllGathers
  fire when half-slabs complete, hidden behind the opposite pass.
- idx/dloc tiles are preloaded once (shared by all 3 layers).
"""
import sys
sys.path.insert(0, '/opt/trn_rl_repo')
import numpy as np
import ml_dtypes

import concourse.bass as bass
import concourse.tile as tile
import concourse.mybir as mybir
from concourse import bacc
from concourse.library_config import mlp as mlp_lib

P = 128
GROUP = 512           # edges per dma_gather (1024-desc ring holds 2/queue)
GCH = GROUP // P      # chunk-cols per group
GPR = GROUP // 256    # pair-chunks per group
SB = 6                # gather groups per sel batch
NQ = 4                # SWDGE queues
LOOKAHEAD = 12        # gather groups in flight per stream phase
ZB = 8                # z~ blocks batched per slab DMA

N_NODES = 50000
N_CORES = 8
DIN = 128
DOUT = 32

np8 = ml_dtypes.float8_e4m3


def _profile_counts(maxq, nblk, slack):
    """Even per-block 128-chunk counts covering maxq+slack edges."""
    need = maxq + slack
    base = max(2, 2 * (need // (256 * nblk)))
    while nblk * base * P > need:
        base -= 2
    base = max(2, base)
    rem = need - nblk * base * P
    ups = min(nblk, (rem + 255) // 256) if rem > 0 else 0
    cnt = np.full(nblk, base, np.int64)
    cnt[:ups] += 2
    return cnt


def _pack_core(indeg, caps, nblk, nodes):
    """Assign `nodes` (list of local ids) to nblk bins of 128 slots with
    per-stream edge caps caps[b][st]. Returns slot_of dict or None."""
    d0 = indeg[nodes, 0].astype(np.int64)
    d1 = indeg[nodes, 1].astype(np.int64)
    order = np.argsort(-(d0 + d1), kind="stable")
    slots_left = np.full(nblk, P, np.int64)
    caps = caps.astype(np.float64)
    used = np.zeros_like(caps)                      # [nblk, 2]
    assign = np.empty(len(nodes), np.int64)
    for i in order:
        dd = np.array([d0[i], d1[i]], np.float64)
        ok = (slots_left > 0) & np.all(caps - used >= dd, axis=1)
        if not ok.any():
            return None
        # most-balanced fill: minimize the worse stream's fill ratio
        ratio = np.max((used + dd) / caps, axis=1)
        ratio[~ok] = 1e18
        b = int(np.argmin(ratio))
        assign[i] = b
        slots_left[b] -= 1
        used[b] += dd
    slot_of = np.empty(len(nodes), np.int64)
    fill = np.zeros(nblk, np.int64)
    for i in range(len(nodes)):
        b = assign[i]
        slot_of[i] = b * P + fill[b]
        fill[b] += 1
    return slot_of


def _preprocess(edge_index, N, C):
    LOCAL = N // C
    NB = (LOCAL + P - 1) // P
    PADL = NB * P
    H0B = NB // 2
    H1B = NB - H0B
    H0R, H1R = H0B * P, H1B * P
    TOT0, TOT1 = C * H0R, C * H1R
    assert TOT0 <= 32768 and TOT1 <= 32768

    src = np.asarray(edge_index[0], dtype=np.int64)
    dst = np.asarray(edge_index[1], dtype=np.int64)

    # degrees INCLUDE self-loops (reference semantics)
    deg = (np.bincount(dst, minlength=N) + 1).astype(np.float64)
    dinv = (1.0 / np.sqrt(deg)).astype(np.float32)
    sdeg = np.sqrt(deg).astype(np.float32)

    score = src // LOCAL
    slocal = src % LOCAL
    s_st = (slocal >= H0R).astype(np.int64)   # source half = stream
    core = dst // LOCAL
    ldst = dst % LOCAL

    # per-core per-node in-degree by stream (excl self-loops)
    indeg = np.zeros((C, LOCAL, 2), np.int64)
    for c in range(C):
        m = core == c
        for st in (0, 1):
            cnts = np.bincount(ldst[m & (s_st == st)], minlength=LOCAL)
            indeg[c, :, st] = cnts

    # shared chunk-count profile per (half, stream) quadrant
    for slack in (256, 512, 1024, 2048):
        cnt = np.zeros((NB, 2), np.int64)
        for h, (lo, b0, nb) in enumerate([(0, 0, H0B), (H0R, H0B, H1B)]):
            hi = lo + (H0R if h == 0 else LOCAL - H0R)
            for st in (0, 1):
                maxq = max(int(indeg[c, lo:hi, st].sum()) for c in range(C))
                cnt[b0:b0 + nb, st] = _profile_counts(maxq, nb, slack)
        # pad each stream's total chunk count to a multiple of GCH
        for st in (0, 1):
            t = int(cnt[:, st].sum())
            i = 0
            while t % GCH:
                cnt[i % NB, st] += 2
                t += 2
                i += 1
        # per-core packing
        slot_of = np.zeros((C, LOCAL), np.int64)
        ok = True
        for c in range(C):
            for h, (lo, b0, nb) in enumerate([(0, 0, H0B), (H0R, H0B, H1B)]):
                hi = lo + (H0R if h == 0 else LOCAL - H0R)
                nodes = np.arange(lo, hi)
                res = _pack_core(indeg[c], cnt[b0:b0 + nb] * P, nb, nodes)
                if res is None:
                    ok = False
                    break
                slot_of[c, nodes] = b0 * P + res
            if not ok:
                break
        if ok:
            break
    assert ok, "bin packing failed at max slack"

    # perm: slot -> orig local id (-1 pad).  half1 has H1R - (LOCAL-H0R)
    # pad slots; they end up wherever packing left slots unused -> compute
    perm = np.full((C, PADL), -1, np.int64)
    for c in range(C):
        perm[c][slot_of[c]] = np.arange(LOCAL)

    # table ids of sources (per the SOURCE core's permutation)
    sslot = slot_of[score, slocal]
    sid = np.where(s_st == 0, score * H0R + sslot,
                   score * H1R + (sslot - H0R))

    n_chunks = [int(cnt[:, st].sum()) for st in (0, 1)]
    n_groups = [n // GCH for n in n_chunks]

    idx_all, dloc_all = [], []
    for c in range(C):
        m = core == c
        cl = slot_of[c][ldst[m]]
        cs, cst = sid[m], s_st[m]
        blk = cl // P
        dl = cl % P
        per_idx, per_dl = [], []
        for st in (0, 1):
            idx_st = np.zeros(n_chunks[st] * P, np.int64)
            dl_st = -np.ones(n_chunks[st] * P, np.int64)
            pos = 0
            for b in range(NB):
                sm = (blk == b) & (cst == st)
                ii, dd = cs[sm], dl[sm]
                want = cnt[b, st] * P
                assert len(ii) <= want, (c, b, st, len(ii), want)
                idx_st[pos:pos + len(ii)] = ii
                dl_st[pos:pos + len(ii)] = dd
                pos += want
            # wrap idx per 1024-group: [G, 128, 64] -> [128, G*64]
            g = n_groups[st]
            w = idx_st.reshape(g, GROUP // 16, 16)
            w = np.transpose(w, (0, 2, 1))             # [g, 16, 64]
            w = np.tile(w, (1, 8, 1))                  # [g, 128, 64]
            per_idx.append(np.ascontiguousarray(
                np.transpose(w, (1, 0, 2)).reshape(P, -1)).astype(np.int16))
            # dloc pairs: [U, 2, 128] -> [128, U*2] f16
            u = n_chunks[st] // 2
            d = dl_st.reshape(u, 2, P)
            per_dl.append(np.ascontiguousarray(
                np.transpose(d, (2, 0, 1)).reshape(P, -1)).astype(np.float16))
        idx_all.append(per_idx)
        dloc_all.append(per_dl)

    return dict(
        LOCAL=LOCAL, NB=NB, PADL=PADL, C=C,
        H0B=H0B, H1B=H1B, TOT0=TOT0, TOT1=TOT1,
        cnt=cnt, n_chunks=n_chunks, n_groups=n_groups,
        idx_all=idx_all, dloc_all=dloc_all, dinv=dinv, sdeg=sdeg, perm=perm,
    )


def _host_tensors(pp, x, weights):
    C, LOCAL, PADL, NB = pp["C"], pp["LOCAL"], pp["PADL"], pp["NB"]
    W1, b1, W2, b2, W3, b3, Wl, bl = weights
    iota = np.tile(np.arange(P, dtype=np.float16), (P, 1))
    ident = np.eye(P, dtype=np.float16)
    ones = np.ones((1, P), np.float16)
    maps = []
    for c in range(C):
        pm = pp["perm"][c]
        valid = pm >= 0
        pmv = np.where(valid, pm, 0)
        xs = np.where(valid[:, None], x[c * LOCAL + pmv], 0).astype(np.float32)
        dvl = np.where(valid, pp["dinv"][c * LOCAL + pmv], 0).astype(np.float32)
        dv = np.ascontiguousarray(dvl.reshape(NB, P).T)
        rd = np.where(valid, pp["sdeg"][c * LOCAL + pmv], 0
                      ).astype(np.float16).reshape(1, -1)
        m = {
            "xt": np.ascontiguousarray(xs.T).astype(np.float16),
            "w1": np.ascontiguousarray(W1, np.float16),
            "w2": np.ascontiguousarray(W2, np.float16),
            "w3": np.ascontiguousarray(W3, np.float16),
            "wl": np.ascontiguousarray(Wl, np.float16),
            "b1": np.asarray(b1, np.float16).reshape(1, -1),
            "b2": np.asarray(b2, np.float16).reshape(1, -1),
            "b3": np.asarray(b3, np.float16).reshape(1, -1),
            "bl": np.asarray(bl, np.float16).reshape(1, -1),
            "dinv_sb": dv, "recipd": rd,
            "iota": iota, "ident": ident, "ones": ones,
        }
        for st in (0, 1):
            m[f"idx{st}"] = pp["idx_all"][c][st]
            m[f"dloc{st}"] = pp["dloc_all"][c][st]
        maps.append(m)
    return maps


def _build(pp, DOUT_, n_cores):
    NB, PADL = pp["NB"], pp["PADL"]
    H0B, H1B, TOT0, TOT1 = pp["H0B"], pp["H1B"], pp["TOT0"], pp["TOT1"]
    cnt, n_groups = pp["cnt"], pp["n_groups"]
    f32 = mybir.dt.float32
    f16 = mybir.dt.float16
    f8 = mybir.dt.float8e4

    nc = bacc.Bacc("TRN2", target_bir_lowering=False, debug=False,
                   num_devices=n_cores, num_swdge_queues=NQ)

    xt = nc.dram_tensor("xt", [P, PADL], f16, kind="ExternalInput")
    w = [nc.dram_tensor(f"w{i+1}", [P, P], f16, kind="ExternalInput") for i in range(3)]
    wl = nc.dram_tensor("wl", [P, DOUT_], f16, kind="ExternalInput")
    bias = [nc.dram_tensor(f"b{i+1}", [1, P], f16, kind="ExternalInput") for i in range(3)]
    bl = nc.dram_tensor("bl", [1, DOUT_], f16, kind="ExternalInput")
    dinv_sb_d = nc.dram_tensor("dinv_sb", [P, NB], f32, kind="ExternalInput")
    recipd_d = nc.dram_tensor("recipd", [1, PADL], f16, kind="ExternalInput")
    iota_d = nc.dram_tensor("iota", [P, P], f16, kind="ExternalInput")
    ident_d = nc.dram_tensor("ident", [P, P], f16, kind="ExternalInput")
    ones_d = nc.dram_tensor("ones", [1, P], f16, kind="ExternalInput")
    idx_d, dloc_d = [None, None], [None, None]
    for st in (0, 1):
        idx_d[st] = nc.dram_tensor(f"idx{st}", [P, n_groups[st] * (GROUP // 16)],
                                   mybir.dt.int16, kind="ExternalInput")
        dloc_d[st] = nc.dram_tensor(f"dloc{st}", [P, n_groups[st] * GPR * 2],
                                    f16, kind="ExternalInput")
    out_d = nc.dram_tensor("out", [PADL, DOUT_], f32, kind="ExternalOutput")

    rg = [list(range(n_cores))]

    from contextlib import ExitStack
    with tile.TileContext(nc) as tc, ExitStack() as ctx:
        dram = ctx.enter_context(tc.tile_pool(name="dram", bufs=1, space="DRAM"))
        cpool = ctx.enter_context(tc.tile_pool(name="consts", bufs=1))
        hpool = ctx.enter_context(tc.tile_pool(name="ht", bufs=1))
        mpool = ctx.enter_context(tc.tile_pool(name="msg", bufs=28))
        spool = ctx.enter_context(tc.tile_pool(name="sel", bufs=3))
        zpool = ctx.enter_context(tc.tile_pool(name="hn", bufs=3))
        opool = ctx.enter_context(tc.tile_pool(name="outs", bufs=3))
        pz = ctx.enter_context(tc.tile_pool(name="pz", bufs=2, space="PSUM"))
        pa = ctx.enter_context(tc.tile_pool(name="pa", bufs=4, space="PSUM"))
        pt = ctx.enter_context(tc.tile_pool(name="pt", bufs=1, space="PSUM"))

        nc.gpsimd.load_library(mlp_lib)

        def const(dram_t, shape, dt=f16):
            t = cpool.tile(shape, dt, name=dram_t.name + "_sb")
            nc.sync.dma_start(t[:], dram_t[:])
            return t
        w_sb = [const(w[i], [P, P]) for i in range(3)]
        wl_sb = const(wl, [P, DOUT_])
        b_sb = [const(bias[i], [1, P]) for i in range(3)]
        bl_sb = const(bl, [1, DOUT_])
        dinv_sb = const(dinv_sb_d, [P, NB], f32)
        recipd = const(recipd_d, [1, PADL])
        iota = const(iota_d, [P, P])
        ident = const(ident_d, [P, P])
        ones = const(ones_d, [1, P])
        idx_sb = [const(idx_d[st], [P, n_groups[st] * (GROUP // 16)],
                        mybir.dt.int16) for st in (0, 1)]
        dloc_sb = [const(dloc_d[st], [P, n_groups[st] * GPR * 2])
                   for st in (0, 1)]

        # persistent state
        hT = [hpool.tile([P, PADL], f16, name=f"hT{i}") for i in range(2)]
        nc.sync.dma_start(hT[0][:], xt[:])
        z8 = [hpool.tile([P, NB * 256], f8, name=f"z8_{i}") for i in range(2)]
        acc = hpool.tile([P, PADL], f16, name="acc")

        slab = [[dram.tile([H0B * P, 256], f8, name=f"slab0_{i}"),
                 dram.tile([H1B * P, 256], f8, name=f"slab1_{i}")]
                for i in range(3)]
        table = [[dram.tile([TOT0, 256], f8, addr_space="Shared",
                            name=f"table0_{i}"),
                  dram.tile([TOT1, 256], f8, addr_space="Shared",
                            name=f"table1_{i}")]
                 for i in range(3)]

        gq = [0]

        zstate = {}

        def z_block(layer, hTsrc, b):
            """Z~ matmul for block b of `layer` -> fp8 dup into z8[layer%2];
            slab DMA every ZB blocks (not straddling the half boundary)."""
            zp = pz.tile([P, P], f32, tag="z")
            nc.tensor.matmul(zp[:], lhsT=hTsrc[:, b * P:(b + 1) * P],
                             rhs=w_sb[layer][:], start=True, stop=True)
            zcur = z8[layer % 2]
            nc.scalar.activation(
                zcur[:, b * 256:(b + 1) * 256].rearrange("p (j e) -> p j e", e=P),
                zp[:].unsqueeze(1).to_broadcast([P, 2, P]),
                mybir.ActivationFunctionType.Copy,
                scale=dinv_sb[:, b:b + 1])
            half = 0 if b < H0B else 1
            bb = b if half == 0 else b - H0B
            nblk = H0B if half == 0 else H1B
            if bb % ZB == 0:
                zstate["b0"] = bb
            if bb % ZB == ZB - 1 or bb == nblk - 1:
                b0 = zstate["b0"]
                nb_ = bb - b0 + 1
                base = (b0 if half == 0 else H0B + b0)
                nc.sync.dma_start(
                    slab[layer][half][b0 * P:(b0 + nb_) * P, :].rearrange(
                        "(g p) f -> p g f", p=P),
                    zcur[:, base * 256:(base + nb_) * 256].rearrange(
                        "p (g f) -> p g f", f=256),
                )

        def ag(layer, half):
            nc.gpsimd.collective_compute(
                "AllGather", mybir.AluOpType.bypass, replica_groups=rg,
                ins=[slab[layer][half].opt()], outs=[table[layer][half].opt()],
            )

        # gather state shared across layers so the NEXT layer's stream-0
        # groups can be pumped during the current layer's tail (hides the
        # gather pipeline refill at layer boundaries)
        g_msg = {}
        g_next = {(l, st): 0 for l in range(3) for st in (0, 1)}

        def pump(layer, st):
            g = g_next[(layer, st)]
            if g >= n_groups[st]:
                return
            g_next[(layer, st)] = g + 1
            mt = mpool.tile([P, GCH * 256], f8, tag="msg")
            nc.gpsimd.dma_gather(
                out_ap=mt[:].rearrange("p (c e) -> p c e", e=256),
                in_ap=table[layer][st][:],
                idxs_ap=idx_sb[st][:, g * (GROUP // 16):(g + 1) * (GROUP // 16)],
                num_idxs=GROUP,
                num_idxs_reg=GROUP,
                elem_size=256,
                queue_num=gq[0] % NQ,
            )
            gq[0] += 1
            g_msg[(layer, st, g)] = mt

        # --- per-layer aggregation ---------------------------------------
        def agg_layer(layer, hTcur, hTnext):
            zcur = z8[layer % 2]

            sel_tiles = [{}, {}]

            def ensure_batch(st, t):
                if t in sel_tiles[st]:
                    return sel_tiles[st][t]
                u0 = t * SB * GPR
                nu = min(SB * GPR, n_groups[st] * GPR - u0)
                st_t = spool.tile([P, SB * GPR * 256], f8, tag="sel")
                nc.vector.tensor_tensor(
                    out=st_t[:, :nu * 256].rearrange(
                        "p (u j s) -> p u j s", j=2, s=P),
                    in0=dloc_sb[st][:, u0 * 2:(u0 + nu) * 2].rearrange(
                        "p (u j) -> p u j", j=2).unsqueeze(3).to_broadcast(
                        [P, nu, 2, P]),
                    in1=iota[:].unsqueeze(1).unsqueeze(1).to_broadcast(
                        [P, nu, 2, P]),
                    op=mybir.AluOpType.is_equal,
                )
                sel_tiles[st][t] = st_t
                return st_t

            cursor = [0, 0]     # pair-chunk cursor per stream

            def emit_pairs(st, b, ap, first):
                """DoubleRow matmuls for block b's stream-st pairs."""
                for _ in range(int(cnt[b, st]) // 2):
                    u = cursor[st]
                    cursor[st] += 1
                    g, j2 = u // GPR, u % GPR
                    mt = g_msg[(layer, st, g)]
                    selt = ensure_batch(st, u // (SB * GPR))
                    k = u - (u // (SB * GPR)) * SB * GPR
                    nc.tensor.matmul(
                        ap[:],
                        lhsT=selt[:, k * 256:(k + 1) * 256].rearrange(
                            "p (j s) -> p j s", s=P),
                        rhs=mt[:].rearrange("p (c e) -> p c e", e=256)[
                            :, 2 * j2:2 * j2 + 2, 0:P],
                        start=first, stop=False,
                        perf_mode=mybir.MatmulPerfMode.DoubleRow,
                    )
                    first = False
                    if j2 == GPR - 1:
                        pump(layer, st)  # group fully claimed; prefetch next
                    if k == SB * GPR - 1:   # last pair of sel batch
                        t = u // (SB * GPR)
                        if (t + 1) * SB * GPR < n_groups[st] * GPR:
                            ensure_batch(st, t + 1)
                return first

            # pass 1: stream-0 partials, spilled (+ self-loop term) to acc
            while g_next[(layer, 0)] < min(LOOKAHEAD, n_groups[0]):
                pump(layer, 0)
            for b in range(NB):
                ap = pa.tile([P, P], f32, tag="agg")
                first = emit_pairs(0, b, ap, True)
                assert not first    # cnt >= 2 guarantees stream-0 pairs
                # acc_b = partial + Z~_local (self-loop), f16
                nc.vector.tensor_tensor(
                    out=acc[:, b * P:(b + 1) * P], in0=ap[:],
                    in1=zcur[:, b * 256:b * 256 + P],
                    op=mybir.AluOpType.add)

            # pass 2: stream-1 + close + activations + next-layer Z~
            while g_next[(layer, 1)] < min(LOOKAHEAD, n_groups[1]):
                pump(layer, 1)
            for b in range(NB):
                ap = pa.tile([P, P], f32, tag="agg")
                first = emit_pairs(1, b, ap, True)
                # close with rank-1 bias (recipd x b), f16
                nc.tensor.matmul(ap[:], lhsT=recipd[0:1, b * P:(b + 1) * P],
                                 rhs=b_sb[layer][:], start=first, stop=True)
                # += stream-0 partial (+self-loop)
                nc.vector.tensor_tensor(
                    out=ap[:], in0=ap[:], in1=acc[:, b * P:(b + 1) * P],
                    op=mybir.AluOpType.add)
                hn = zpool.tile([P, P], f16, tag="hn")
                nc.scalar.activation(hn[:], ap[:],
                                     mybir.ActivationFunctionType.Relu,
                                     scale=dinv_sb[:, b:b + 1])
                tp = pt.tile([P, P], f16, tag="tp")
                nc.tensor.transpose(out=tp[:], in_=hn[:], identity=ident[:])
                nc.vector.tensor_copy(hTnext[:, b * P:(b + 1) * P], tp[:])
                if layer < 2:
                    z_block(layer + 1, hTnext, b)
                    if b == H0B - 1:
                        ag(layer + 1, 0)
                    elif b == NB - 1:
                        ag(layer + 1, 1)
                else:
                    hp = pz.tile([P, DOUT_], f32, tag="z")
                    nc.tensor.matmul(hp[:], lhsT=hTnext[:, b * P:(b + 1) * P],
                                     rhs=wl_sb[:], start=True, stop=False)
                    nc.tensor.matmul(hp[:], lhsT=ones[:], rhs=bl_sb[:],
                                     start=False, stop=True)
                    ot = opool.tile([P, DOUT_], f32, tag="o")
                    nc.scalar.activation(ot[:], hp[:],
                                         mybir.ActivationFunctionType.Copy)
                    nc.sync.dma_start(out_d[b * P:(b + 1) * P, :], ot[:])

            # cross-layer warmup: issue the next layer's stream-0 gathers now
            # (they park on the AG0 sem while this layer's PE tail drains)
            if layer < 2:
                for _ in range(min(LOOKAHEAD, n_groups[0])):
                    pump(layer + 1, 0)

        # prologue: layer-0 Z~ from x, both AGs
        for b in range(NB):
            z_block(0, hT[0], b)
            if b == H0B - 1:
                ag(0, 0)
            elif b == NB - 1:
                ag(0, 1)

        for layer in range(3):
            agg_layer(layer, hT[layer % 2], hT[(layer + 1) % 2])

    nc.compile()
    return nc


_CACHE = {}


def _get_compiled(edge_index):
    key = hash(np.asarray(edge_index, np.int64).tobytes())
    if key not in _CACHE:
        pp = _preprocess(edge_index, N_NODES, N_CORES)
        nc = _build(pp, DOUT, N_CORES)
        _CACHE[key] = (pp, nc)
    return _CACHE[key]


_LAST_RUN = {}


def kernel(x, edge_index, W1, b1, W2, b2, W3, b3, Wl, bl):
    x = np.asarray(x, np.float32)
    pp, nc = _get_compiled(edge_index)
    maps = _host_tensors(pp, x, (W1, b1, W2, b2, W3, b3, Wl, bl))

    from concourse.bass_utils import run_bass_kernel_spmd
    res = run_bass_kernel_spmd(nc, maps, core_ids=list(range(N_CORES)))
    LOCAL = pp["LOCAL"]
    parts = []
    for c in range(N_CORES):
        r = np.asarray(res.results[c]["out"])
        pm = pp["perm"][c]
        valid = pm >= 0
        o = np.zeros((LOCAL, r.shape[1]), r.dtype)
        o[pm[valid]] = r[valid]
        parts.append(o)
    out = np.concatenate(parts)
    _LAST_RUN["nc"] = nc
    _LAST_RUN["maps"] = maps
    return out


def _install_ntff_hook():
    """The agent image's antenv lacks axon_hooks; recreate it from the boot
    helper so run_bass_kernel_spmd(trace=True) can capture NTFF profiles."""
    import types
    if "antenv.axon_hooks" in sys.modules:
        return
    mod = types.ModuleType("antenv.axon_hooks")
    _state = {}
    mod.set_axon_ntff_profile_hook = lambda h: _state.__setitem__("h", h)
    mod.get_axon_ntff_profile_hook = lambda: _state.get("h")
    sys.modules["antenv.axon_hooks"] = mod
    import antenv
    antenv.axon_hooks = mod
    from trn_agent_boot.trn_boot import _ntff_profile_via_ctypes
    mod.set_axon_ntff_profile_hook(
        _ntff_profile_via_ctypes("/opt/axon/libaxon_pjrt.so"))


def profile_exec_ns():
    """Re-run the last kernel invocation with NTFF tracing; return exec ns."""
    if "nc" not in _LAST_RUN:
        return None
    _install_ntff_hook()
    from concourse.bass_utils import run_bass_kernel_spmd
    res = run_bass_kernel_spmd(
        _LAST_RUN["nc"], _LAST_RUN["maps"],
        core_ids=list(range(N_CORES)), trace=True,
    )
    _LAST_RUN["trace_res"] = res
    return res.exec_time_ns


# revision 27
# speedup vs baseline: 1.0225x; 1.0172x over previous
"""3-layer GCN + linear head on 8 TRN2 NeuronCores (Bass/Tile, SPMD).

Self-contained: hardcodes N=50000, E=600000, D=128, DOUT=32, 8 cores.

Math (matches the reference):
    src,dst + self-loops; deg = in-degree incl self-loop; dinv = rsqrt(deg)
    norm_e = dinv[src]*dinv[dst]
    layer(h): agg[d] = sum_e norm_e (hW)[src_e]; relu(agg+b)
    out = h3 @ Wl + bl

v2 design:
- Nodes sharded into 8 slabs (graph parallel); per layer Z~ = dinv (H W) is
  written as an fp8e4m3 DUPLICATED-row slab ([row;row] = 256B, the SWDGE
  gather minimum) and AllGathered into a shared table per half.
- Self-loops are NOT in the edge stream: Z~_local is added by DVE directly.
- Edges (w/o loops) are packed host-side into 128-edge chunks per
  (dst-block, src-half) with a SHARED even chunk-count profile; per-core
  cap-constrained bin-packing of nodes into blocks keeps padding ~1-3%.
- Gather: 1024-edge SWDGE groups (ring limit) of 256B fp8 rows, 4 queues.
- Aggregation: fp8 DoubleRow matmuls — 256 edges (2 chunks) per PE
  instruction: lhsT = one-hot sel [128,2,128] fp8 (DVE is_equal), rhs = msg
  [128,2,128] fp8 -> PSUM f32. Bias enters as a f16 rank-1 matmul
  (recipd x b); dinv scaling + relu on Act.
- Two passes per layer over the 49 blocks: pass1 accumulates stream-0
  chunks and spills acc_b = PSUM + Z~_local(self-loop) to SBUF f16 (DVE);
  pass2 accumulates stream-1 chunks, DVE-adds acc_b, closes with the bias
  matmul, then relu -> PE transpose -> next layer's Z~ per block. AllGathers
  fire when half-slabs complete, hidden behind the opposite pass.
- idx/dloc tiles are preloaded once (shared by all 3 layers).
"""
import sys
sys.path.insert(0, '/opt/trn_rl_repo')
import numpy as np
import ml_dtypes

import concourse.bass as bass
import concourse.tile as tile
import concourse.mybir as mybir
from concourse import bacc
from concourse.library_config import mlp as mlp_lib

P = 128
GROUP = 512           # edges per dma_gather (1024-desc ring holds 2/queue)
GCH = GROUP // P      # chunk-cols per group
GPR = GROUP // 256    # pair-chunks per group
SB = 6                # gather groups per sel batch
NQ = 4                # SWDGE queues
LOOKAHEAD = 16        # gather groups in flight per stream phase
ZB = 8                # z~ blocks batched per slab DMA

N_NODES = 50000
N_CORES = 8
DIN = 128
DOUT = 32

np8 = ml_dtypes.float8_e4m3


def _profile_counts(maxq, nblk, slack):
    """Even per-block 128-chunk counts covering maxq+slack edges."""
    need = maxq + slack
    base = max(2, 2 * (need // (256 * nblk)))
    while nblk * base * P > need:
        base -= 2
    base = max(2, base)
    rem = need - nblk * base * P
    ups = min(nblk, (rem + 255) // 256) if rem > 0 else 0
    cnt = np.full(nblk, base, np.int64)
    cnt[:ups] += 2
    return cnt


def _pack_core(indeg, caps, nblk, nodes):
    """Assign `nodes` (list of local ids) to nblk bins of 128 slots with
    per-stream edge caps caps[b][st]. Returns slot_of dict or None."""
    d0 = indeg[nodes, 0].astype(np.int64)
    d1 = indeg[nodes, 1].astype(np.int64)
    order = np.argsort(-(d0 + d1), kind="stable")
    slots_left = np.full(nblk, P, np.int64)
    caps = caps.astype(np.float64)
    used = np.zeros_like(caps)                      # [nblk, 2]
    assign = np.empty(len(nodes), np.int64)
    for i in order:
        dd = np.array([d0[i], d1[i]], np.float64)
        ok = (slots_left > 0) & np.all(caps - used >= dd, axis=1)
        if not ok.any():
            return None
        # most-balanced fill: minimize the worse stream's fill ratio
        ratio = np.max((used + dd) / caps, axis=1)
        ratio[~ok] = 1e18
        b = int(np.argmin(ratio))
        assign[i] = b
        slots_left[b] -= 1
        used[b] += dd
    slot_of = np.empty(len(nodes), np.int64)
    fill = np.zeros(nblk, np.int64)
    for i in range(len(nodes)):
        b = assign[i]
        slot_of[i] = b * P + fill[b]
        fill[b] += 1
    return slot_of


def _preprocess(edge_index, N, C):
    LOCAL = N // C
    NB = (LOCAL + P - 1) // P
    PADL = NB * P
    H0B = NB // 2
    H1B = NB - H0B
    H0R, H1R = H0B * P, H1B * P
    TOT0, TOT1 = C * H0R, C * H1R
    assert TOT0 <= 32768 and TOT1 <= 32768

    src = np.asarray(edge_index[0], dtype=np.int64)
    dst = np.asarray(edge_index[1], dtype=np.int64)

    # degrees INCLUDE self-loops (reference semantics)
    deg = (np.bincount(dst, minlength=N) + 1).astype(np.float64)
    dinv = (1.0 / np.sqrt(deg)).astype(np.float32)
    sdeg = np.sqrt(deg).astype(np.float32)

    score = src // LOCAL
    slocal = src % LOCAL
    s_st = (slocal >= H0R).astype(np.int64)   # source half = stream
    core = dst // LOCAL
    ldst = dst % LOCAL

    # per-core per-node in-degree by stream (excl self-loops)
    indeg = np.zeros((C, LOCAL, 2), np.int64)
    for c in range(C):
        m = core == c
        for st in (0, 1):
            cnts = np.bincount(ldst[m & (s_st == st)], minlength=LOCAL)
            indeg[c, :, st] = cnts

    # shared chunk-count profile per (half, stream) quadrant
    for slack in (256, 512, 1024, 2048):
        cnt = np.zeros((NB, 2), np.int64)
        for h, (lo, b0, nb) in enumerate([(0, 0, H0B), (H0R, H0B, H1B)]):
            hi = lo + (H0R if h == 0 else LOCAL - H0R)
            for st in (0, 1):
                maxq = max(int(indeg[c, lo:hi, st].sum()) for c in range(C))
                cnt[b0:b0 + nb, st] = _profile_counts(maxq, nb, slack)
        # pad each stream's total chunk count to a multiple of GCH
        for st in (0, 1):
            t = int(cnt[:, st].sum())
            i = 0
            while t % GCH:
                cnt[i % NB, st] += 2
                t += 2
                i += 1
        # per-core packing
        slot_of = np.zeros((C, LOCAL), np.int64)
        ok = True
        for c in range(C):
            for h, (lo, b0, nb) in enumerate([(0, 0, H0B), (H0R, H0B, H1B)]):
                hi = lo + (H0R if h == 0 else LOCAL - H0R)
                nodes = np.arange(lo, hi)
                res = _pack_core(indeg[c], cnt[b0:b0 + nb] * P, nb, nodes)
                if res is None:
                    ok = False
                    break
                slot_of[c, nodes] = b0 * P + res
            if not ok:
                break
        if ok:
            break
    assert ok, "bin packing failed at max slack"

    # perm: slot -> orig local id (-1 pad).  half1 has H1R - (LOCAL-H0R)
    # pad slots; they end up wherever packing left slots unused -> compute
    perm = np.full((C, PADL), -1, np.int64)
    for c in range(C):
        perm[c][slot_of[c]] = np.arange(LOCAL)

    # table ids of sources (per the SOURCE core's permutation)
    sslot = slot_of[score, slocal]
    sid = np.where(s_st == 0, score * H0R + sslot,
                   score * H1R + (sslot - H0R))

    n_chunks = [int(cnt[:, st].sum()) for st in (0, 1)]
    n_groups = [n // GCH for n in n_chunks]

    idx_all, dloc_all = [], []
    for c in range(C):
        m = core == c
        cl = slot_of[c][ldst[m]]
        cs, cst = sid[m], s_st[m]
        blk = cl // P
        dl = cl % P
        per_idx, per_dl = [], []
        for st in (0, 1):
            idx_st = np.zeros(n_chunks[st] * P, np.int64)
            dl_st = -np.ones(n_chunks[st] * P, np.int64)
            pos = 0
            for b in range(NB):
                sm = (blk == b) & (cst == st)
                ii, dd = cs[sm], dl[sm]
                want = cnt[b, st] * P
                assert len(ii) <= want, (c, b, st, len(ii), want)
                idx_st[pos:pos + len(ii)] = ii
                dl_st[pos:pos + len(ii)] = dd
                pos += want
            # wrap idx per 1024-group: [G, 128, 64] -> [128, G*64]
            g = n_groups[st]
            w = idx_st.reshape(g, GROUP // 16, 16)
            w = np.transpose(w, (0, 2, 1))             # [g, 16, 64]
            w = np.tile(w, (1, 8, 1))                  # [g, 128, 64]
            per_idx.append(np.ascontiguousarray(
                np.transpose(w, (1, 0, 2)).reshape(P, -1)).astype(np.int16))
            # dloc pairs: [U, 2, 128] -> [128, U*2] f16
            u = n_chunks[st] // 2
            d = dl_st.reshape(u, 2, P)
            per_dl.append(np.ascontiguousarray(
                np.transpose(d, (2, 0, 1)).reshape(P, -1)).astype(np.float16))
        idx_all.append(per_idx)
        dloc_all.append(per_dl)

    return dict(
        LOCAL=LOCAL, NB=NB, PADL=PADL, C=C,
        H0B=H0B, H1B=H1B, TOT0=TOT0, TOT1=TOT1,
        cnt=cnt, n_chunks=n_chunks, n_groups=n_groups,
        idx_all=idx_all, dloc_all=dloc_all, dinv=dinv, sdeg=sdeg, perm=perm,
    )


def _host_tensors(pp, x, weights):
    C, LOCAL, PADL, NB = pp["C"], pp["LOCAL"], pp["PADL"], pp["NB"]
    W1, b1, W2, b2, W3, b3, Wl, bl = weights
    iota = np.tile(np.arange(P, dtype=np.float16), (P, 1))
    ident = np.eye(P, dtype=np.float16)
    ones = np.ones((1, P), np.float16)
    maps = []
    for c in range(C):
        pm = pp["perm"][c]
        valid = pm >= 0
        pmv = np.where(valid, pm, 0)
        xs = np.where(valid[:, None], x[c * LOCAL + pmv], 0).astype(np.float32)
        dvl = np.where(valid, pp["dinv"][c * LOCAL + pmv], 0).astype(np.float32)
        dv = np.ascontiguousarray(dvl.reshape(NB, P).T)
        rd = np.where(valid, pp["sdeg"][c * LOCAL + pmv], 0
                      ).astype(np.float16).reshape(1, -1)
        m = {
            "xt": np.ascontiguousarray(xs.T).astype(np.float16),
            "w1": np.ascontiguousarray(W1, np.float16),
            "w2": np.ascontiguousarray(W2, np.float16),
            "w3": np.ascontiguousarray(W3, np.float16),
            "wl": np.ascontiguousarray(Wl, np.float16),
            "b1": np.asarray(b1, np.float16).reshape(1, -1),
            "b2": np.asarray(b2, np.float16).reshape(1, -1),
            "b3": np.asarray(b3, np.float16).reshape(1, -1),
            "bl": np.asarray(bl, np.float16).reshape(1, -1),
            "dinv_sb": dv, "recipd": rd,
            "iota": iota, "ident": ident, "ones": ones,
        }
        for st in (0, 1):
            m[f"idx{st}"] = pp["idx_all"][c][st]
            m[f"dloc{st}"] = pp["dloc_all"][c][st]
        maps.append(m)
    return maps


def _build(pp, DOUT_, n_cores):
    NB, PADL = pp["NB"], pp["PADL"]
    H0B, H1B, TOT0, TOT1 = pp["H0B"], pp["H1B"], pp["TOT0"], pp["TOT1"]
    cnt, n_groups = pp["cnt"], pp["n_groups"]
    f32 = mybir.dt.float32
    f16 = mybir.dt.float16
    f8 = mybir.dt.float8e4

    nc = bacc.Bacc("TRN2", target_bir_lowering=False, debug=False,
                   num_devices=n_cores, num_swdge_queues=NQ)

    xt = nc.dram_tensor("xt", [P, PADL], f16, kind="ExternalInput")
    w = [nc.dram_tensor(f"w{i+1}", [P, P], f16, kind="ExternalInput") for i in range(3)]
    wl = nc.dram_tensor("wl", [P, DOUT_], f16, kind="ExternalInput")
    bias = [nc.dram_tensor(f"b{i+1}", [1, P], f16, kind="ExternalInput") for i in range(3)]
    bl = nc.dram_tensor("bl", [1, DOUT_], f16, kind="ExternalInput")
    dinv_sb_d = nc.dram_tensor("dinv_sb", [P, NB], f32, kind="ExternalInput")
    recipd_d = nc.dram_tensor("recipd", [1, PADL], f16, kind="ExternalInput")
    iota_d = nc.dram_tensor("iota", [P, P], f16, kind="ExternalInput")
    ident_d = nc.dram_tensor("ident", [P, P], f16, kind="ExternalInput")
    ones_d = nc.dram_tensor("ones", [1, P], f16, kind="ExternalInput")
    idx_d, dloc_d = [None, None], [None, None]
    for st in (0, 1):
        idx_d[st] = nc.dram_tensor(f"idx{st}", [P, n_groups[st] * (GROUP // 16)],
                                   mybir.dt.int16, kind="ExternalInput")
        dloc_d[st] = nc.dram_tensor(f"dloc{st}", [P, n_groups[st] * GPR * 2],
                                    f16, kind="ExternalInput")
    out_d = nc.dram_tensor("out", [PADL, DOUT_], f32, kind="ExternalOutput")

    rg = [list(range(n_cores))]

    from contextlib import ExitStack
    with tile.TileContext(nc) as tc, ExitStack() as ctx:
        dram = ctx.enter_context(tc.tile_pool(name="dram", bufs=1, space="DRAM"))
        cpool = ctx.enter_context(tc.tile_pool(name="consts", bufs=1))
        hpool = ctx.enter_context(tc.tile_pool(name="ht", bufs=1))
        mpool = ctx.enter_context(tc.tile_pool(name="msg", bufs=28))
        spool = ctx.enter_context(tc.tile_pool(name="sel", bufs=3))
        zpool = ctx.enter_context(tc.tile_pool(name="hn", bufs=3))
        opool = ctx.enter_context(tc.tile_pool(name="outs", bufs=3))
        pz = ctx.enter_context(tc.tile_pool(name="pz", bufs=2, space="PSUM"))
        pa = ctx.enter_context(tc.tile_pool(name="pa", bufs=4, space="PSUM"))
        pt = ctx.enter_context(tc.tile_pool(name="pt", bufs=1, space="PSUM"))

        nc.gpsimd.load_library(mlp_lib)

        def const(dram_t, shape, dt=f16):
            t = cpool.tile(shape, dt, name=dram_t.name + "_sb")
            nc.sync.dma_start(t[:], dram_t[:])
            return t
        w_sb = [const(w[i], [P, P]) for i in range(3)]
        wl_sb = const(wl, [P, DOUT_])
        b_sb = [const(bias[i], [1, P]) for i in range(3)]
        bl_sb = const(bl, [1, DOUT_])
        dinv_sb = const(dinv_sb_d, [P, NB], f32)
        recipd = const(recipd_d, [1, PADL])
        iota = const(iota_d, [P, P])
        ident = const(ident_d, [P, P])
        ones = const(ones_d, [1, P])
        idx_sb = [const(idx_d[st], [P, n_groups[st] * (GROUP // 16)],
                        mybir.dt.int16) for st in (0, 1)]
        dloc_sb = [const(dloc_d[st], [P, n_groups[st] * GPR * 2])
                   for st in (0, 1)]

        # persistent state
        hT = [hpool.tile([P, PADL], f16, name=f"hT{i}") for i in range(2)]
        nc.sync.dma_start(hT[0][:], xt[:])
        z8 = [hpool.tile([P, NB * 256], f8, name=f"z8_{i}") for i in range(2)]
        acc = hpool.tile([P, PADL], f16, name="acc")

        slab = [[dram.tile([H0B * P, 256], f8, name=f"slab0_{i}"),
                 dram.tile([H1B * P, 256], f8, name=f"slab1_{i}")]
                for i in range(3)]
        table = [[dram.tile([TOT0, 256], f8, addr_space="Shared",
                            name=f"table0_{i}"),
                  dram.tile([TOT1, 256], f8, addr_space="Shared",
                            name=f"table1_{i}")]
                 for i in range(3)]

        gq = [0]

        zstate = {}

        def z_block(layer, hTsrc, b):
            """Z~ matmul for block b of `layer` -> fp8 dup into z8[layer%2];
            slab DMA every ZB blocks (not straddling the half boundary)."""
            zp = pz.tile([P, P], f32, tag="z")
            nc.tensor.matmul(zp[:], lhsT=hTsrc[:, b * P:(b + 1) * P],
                             rhs=w_sb[layer][:], start=True, stop=True)
            zcur = z8[layer % 2]
            nc.scalar.activation(
                zcur[:, b * 256:(b + 1) * 256].rearrange("p (j e) -> p j e", e=P),
                zp[:].unsqueeze(1).to_broadcast([P, 2, P]),
                mybir.ActivationFunctionType.Copy,
                scale=dinv_sb[:, b:b + 1])
            half = 0 if b < H0B else 1
            bb = b if half == 0 else b - H0B
            nblk = H0B if half == 0 else H1B
            if bb % ZB == 0:
                zstate["b0"] = bb
            if bb % ZB == ZB - 1 or bb == nblk - 1:
                b0 = zstate["b0"]
                nb_ = bb - b0 + 1
                base = (b0 if half == 0 else H0B + b0)
                nc.sync.dma_start(
                    slab[layer][half][b0 * P:(b0 + nb_) * P, :].rearrange(
                        "(g p) f -> p g f", p=P),
                    zcur[:, base * 256:(base + nb_) * 256].rearrange(
                        "p (g f) -> p g f", f=256),
                )

        def ag(layer, half):
            nc.gpsimd.collective_compute(
                "AllGather", mybir.AluOpType.bypass, replica_groups=rg,
                ins=[slab[layer][half].opt()], outs=[table[layer][half].opt()],
            )

        # gather state shared across layers so the NEXT layer's stream-0
        # groups can be pumped during the current layer's tail (hides the
        # gather pipeline refill at layer boundaries)
        g_msg = {}
        g_next = {(l, st): 0 for l in range(3) for st in (0, 1)}

        def pump(layer, st):
            g = g_next[(layer, st)]
            if g >= n_groups[st]:
                return
            g_next[(layer, st)] = g + 1
            mt = mpool.tile([P, GCH * 256], f8, tag="msg")
            nc.gpsimd.dma_gather(
                out_ap=mt[:].rearrange("p (c e) -> p c e", e=256),
                in_ap=table[layer][st][:],
                idxs_ap=idx_sb[st][:, g * (GROUP // 16):(g + 1) * (GROUP // 16)],
                num_idxs=GROUP,
                num_idxs_reg=GROUP,
                elem_size=256,
                queue_num=gq[0] % NQ,
            )
            gq[0] += 1
            g_msg[(layer, st, g)] = mt

        # --- per-layer aggregation ---------------------------------------
        def agg_layer(layer, hTcur, hTnext):
            zcur = z8[layer % 2]

            sel_tiles = [{}, {}]

            def ensure_batch(st, t):
                if t in sel_tiles[st]:
                    return sel_tiles[st][t]
                u0 = t * SB * GPR
                nu = min(SB * GPR, n_groups[st] * GPR - u0)
                st_t = spool.tile([P, SB * GPR * 256], f8, tag="sel")
                nc.vector.tensor_tensor(
                    out=st_t[:, :nu * 256].rearrange(
                        "p (u j s) -> p u j s", j=2, s=P),
                    in0=dloc_sb[st][:, u0 * 2:(u0 + nu) * 2].rearrange(
                        "p (u j) -> p u j", j=2).unsqueeze(3).to_broadcast(
                        [P, nu, 2, P]),
                    in1=iota[:].unsqueeze(1).unsqueeze(1).to_broadcast(
                        [P, nu, 2, P]),
                    op=mybir.AluOpType.is_equal,
                )
                sel_tiles[st][t] = st_t
                return st_t

            cursor = [0, 0]     # pair-chunk cursor per stream

            def emit_pairs(st, b, ap, first):
                """DoubleRow matmuls for block b's stream-st pairs."""
                for _ in range(int(cnt[b, st]) // 2):
                    u = cursor[st]
                    cursor[st] += 1
                    g, j2 = u // GPR, u % GPR
                    mt = g_msg[(layer, st, g)]
                    selt = ensure_batch(st, u // (SB * GPR))
                    k = u - (u // (SB * GPR)) * SB * GPR
                    nc.tensor.matmul(
                        ap[:],
                        lhsT=selt[:, k * 256:(k + 1) * 256].rearrange(
                            "p (j s) -> p j s", s=P),
                        rhs=mt[:].rearrange("p (c e) -> p c e", e=256)[
                            :, 2 * j2:2 * j2 + 2, 0:P],
                        start=first, stop=False,
                        perf_mode=mybir.MatmulPerfMode.DoubleRow,
                    )
                    first = False
                    if j2 == GPR - 1:
                        pump(layer, st)  # group fully claimed; prefetch next
                    if k == SB * GPR - 1:   # last pair of sel batch
                        t = u // (SB * GPR)
                        if (t + 1) * SB * GPR < n_groups[st] * GPR:
                            ensure_batch(st, t + 1)
                return first

            # pass 1: stream-0 partials, spilled (+ self-loop term) to acc
            while g_next[(layer, 0)] < min(LOOKAHEAD, n_groups[0]):
                pump(layer, 0)
            for b in range(NB):
                ap = pa.tile([P, P], f32, tag="agg")
                first = emit_pairs(0, b, ap, True)
                assert not first    # cnt >= 2 guarantees stream-0 pairs
                # acc_b = partial + Z~_local (self-loop), f16
                nc.vector.tensor_tensor(
                    out=acc[:, b * P:(b + 1) * P], in0=ap[:],
                    in1=zcur[:, b * 256:b * 256 + P],
                    op=mybir.AluOpType.add)

            # pass 2: stream-1 + close + activations + next-layer Z~
            while g_next[(layer, 1)] < min(LOOKAHEAD, n_groups[1]):
                pump(layer, 1)
            for b in range(NB):
                ap = pa.tile([P, P], f32, tag="agg")
                first = emit_pairs(1, b, ap, True)
                # close with rank-1 bias (recipd x b), f16
                nc.tensor.matmul(ap[:], lhsT=recipd[0:1, b * P:(b + 1) * P],
                                 rhs=b_sb[layer][:], start=first, stop=True)
                # += stream-0 partial (+self-loop)
                nc.vector.tensor_tensor(
                    out=ap[:], in0=ap[:], in1=acc[:, b * P:(b + 1) * P],
                    op=mybir.AluOpType.add)
                hn = zpool.tile([P, P], f16, tag="hn")
                nc.scalar.activation(hn[:], ap[:],
                                     mybir.ActivationFunctionType.Relu,
                                     scale=dinv_sb[:, b:b + 1])
                tp = pt.tile([P, P], f16, tag="tp")
                nc.tensor.transpose(out=tp[:], in_=hn[:], identity=ident[:])
                nc.vector.tensor_copy(hTnext[:, b * P:(b + 1) * P], tp[:])
                if layer < 2:
                    z_block(layer + 1, hTnext, b)
                    if b == H0B - 1:
                        ag(layer + 1, 0)
                    elif b == NB - 1:
                        ag(layer + 1, 1)
                else:
                    hp = pz.tile([P, DOUT_], f32, tag="z")
                    nc.tensor.matmul(hp[:], lhsT=hTnext[:, b * P:(b + 1) * P],
                                     rhs=wl_sb[:], start=True, stop=False)
                    nc.tensor.matmul(hp[:], lhsT=ones[:], rhs=bl_sb[:],
                                     start=False, stop=True)
                    ot = opool.tile([P, DOUT_], f32, tag="o")
                    nc.scalar.activation(ot[:], hp[:],
                                         mybir.ActivationFunctionType.Copy)
                    nc.sync.dma_start(out_d[b * P:(b + 1) * P, :], ot[:])

            # cross-layer warmup: issue the next layer's stream-0 gathers now
            # (they park on the AG0 sem while this layer's PE tail drains)
            if layer < 2:
                for _ in range(min(LOOKAHEAD, n_groups[0])):
                    pump(layer + 1, 0)

        # prologue: layer-0 Z~ from x, both AGs
        for b in range(NB):
            z_block(0, hT[0], b)
            if b == H0B - 1:
                ag(0, 0)
            elif b == NB - 1:
                ag(0, 1)

        for layer in range(3):
            agg_layer(layer, hT[layer % 2], hT[(layer + 1) % 2])

    nc.compile()
    return nc


_CACHE = {}


def _get_compiled(edge_index):
    key = hash(np.asarray(edge_index, np.int64).tobytes())
    if key not in _CACHE:
        pp = _preprocess(edge_index, N_NODES, N_CORES)
        nc = _build(pp, DOUT, N_CORES)
        _CACHE[key] = (pp, nc)
    return _CACHE[key]


_LAST_RUN = {}


def kernel(x, edge_index, W1, b1, W2, b2, W3, b3, Wl, bl):
    x = np.asarray(x, np.float32)
    pp, nc = _get_compiled(edge_index)
    maps = _host_tensors(pp, x, (W1, b1, W2, b2, W3, b3, Wl, bl))

    from concourse.bass_utils import run_bass_kernel_spmd
    res = run_bass_kernel_spmd(nc, maps, core_ids=list(range(N_CORES)))
    LOCAL = pp["LOCAL"]
    parts = []
    for c in range(N_CORES):
        r = np.asarray(res.results[c]["out"])
        pm = pp["perm"][c]
        valid = pm >= 0
        o = np.zeros((LOCAL, r.shape[1]), r.dtype)
        o[pm[valid]] = r[valid]
        parts.append(o)
    out = np.concatenate(parts)
    _LAST_RUN["nc"] = nc
    _LAST_RUN["maps"] = maps
    return out


def _install_ntff_hook():
    """The agent image's antenv lacks axon_hooks; recreate it from the boot
    helper so run_bass_kernel_spmd(trace=True) can capture NTFF profiles."""
    import types
    if "antenv.axon_hooks" in sys.modules:
        return
    mod = types.ModuleType("antenv.axon_hooks")
    _state = {}
    mod.set_axon_ntff_profile_hook = lambda h: _state.__setitem__("h", h)
    mod.get_axon_ntff_profile_hook = lambda: _state.get("h")
    sys.modules["antenv.axon_hooks"] = mod
    import antenv
    antenv.axon_hooks = mod
    from trn_agent_boot.trn_boot import _ntff_profile_via_ctypes
    mod.set_axon_ntff_profile_hook(
        _ntff_profile_via_ctypes("/opt/axon/libaxon_pjrt.so"))


def profile_exec_ns():
    """Re-run the last kernel invocation with NTFF tracing; return exec ns."""
    if "nc" not in _LAST_RUN:
        return None
    _install_ntff_hook()
    from concourse.bass_utils import run_bass_kernel_spmd
    res = run_bass_kernel_spmd(
        _LAST_RUN["nc"], _LAST_RUN["maps"],
        core_ids=list(range(N_CORES)), trace=True,
    )
    _LAST_RUN["trace_res"] = res
    return res.exec_time_ns
